# revision 2
# baseline (speedup 1.0000x reference)
"""BiRNN (last-hidden) Trainium2 kernel, 8 NeuronCores.

Problem: B,T,I,H,C = 64,512,256,512,128
  xf = x @ Wxf^T + bxf ; hf = scan tanh(xf_t + h Whf^T + bhf)
  xb = x @ Wxb^T + bxb ; hb = reverse scan
  out = [hf|hb] @ fc_w^T + fc_b

Sharding: cores 0-3 forward direction, batch slices of 16; cores 4-7
backward direction (x time-reversed on host), same batch slices.
No collectives: each core emits a partial fc product [C, 16]; the host
adds fwd+bwd partials and fc_b.

Per-core program (raw Bass, explicit semaphores):
  phase 1  GEMM: xw[j,(t,b)] = WxT.T @ xT (+ bxf+bhf folded via ACT bias)
  phase 2  512 sequential steps; h kept as [j(4x128 part), b] so the
           recurrent matmuls need no transposes; 16 MMs/step with WhhT
           tiles stationary; DVE adds psum+xw, ACT applies tanh.
  phase 3  fc: 4 matmuls -> [C, BL] partial output.
All operand transposes are done host-side in numpy.
"""

import sys
from contextlib import ExitStack

sys.path.insert(0, "/opt/trn_rl_repo")

import numpy as np

import concourse.bass as bass
from concourse import mybir
from concourse.bass_utils import run_bass_kernel_spmd

B, T, I, H, C = 64, 512, 256, 512, 128
NCORES = 8
BL = 16          # batch rows per core
IC = I // 128    # 2 contraction chunks for the input GEMM
KC = H // 128    # 4 contraction / output chunks for the recurrence
CHUNK = 512      # GEMM column chunk (columns are (t, b) pairs)
NXBUF = 4        # x streaming buffers
NGBANK = 3       # psum banks for GEMM epilogue pipelining


QUARTERS = (((0, 1), (0, 1)), ((2, 3), (0, 1)), ((0, 1), (2, 3)), ((2, 3), (2, 3)))


def build_nc(t_steps=T, dt_gemm=mybir.dt.float32, dt_rec=mybir.dt.float32,
             variant="full", reps=1):
    nc = bass.Bass()
    f32 = mybir.dt.float32
    ncols = t_steps * BL
    nchunks = ncols // CHUNK
    assert nchunks * CHUNK == ncols
    tpc = CHUNK // BL
    nunits = nchunks * KC
    rec = variant in ("full", "nodep")
    TS = t_steps if rec else 0

    # per-rep semaphore totals
    TOT_dma_w = 80
    TOT_dma_x = [32 * len(range(sl, nchunks, NXBUF)) for sl in range(NXBUF)]
    TOT_pe_g = nunits
    TOT_act_g = nunits // 2
    TOT_dve_g = nunits // 2
    TOT_pe_s = 2 * TS + 1
    TOT_dve_s = 2 * TS + 1
    TOT_act_s = 2 * TS
    TOT_fc_s = 1

    xT = nc.declare_dram_parameter("xT", [IC, 128, ncols], dt_gemm, isOutput=False)
    WxT = nc.declare_dram_parameter("WxT", [IC, 128, H], dt_gemm, isOutput=False)
    WhT = nc.declare_dram_parameter("WhT", [KC, 128, H], dt_rec, isOutput=False)
    bias = nc.declare_dram_parameter("bias", [128, KC], f32, isOutput=False)
    fcwT = nc.declare_dram_parameter("fcwT", [KC, 128, C], dt_rec, isOutput=False)
    out = nc.declare_dram_parameter("out", [C, BL], f32, isOutput=True)

    with ExitStack() as ctx:
        ec = ctx.enter_context
        sb_WxT = ec(nc.sbuf_tensor([128, IC, H], dt_gemm))
        sb_WhT = ec(nc.sbuf_tensor([128, KC, H], dt_rec))
        sb_bias = ec(nc.sbuf_tensor([128, KC], f32))
        sb_fcwT = ec(nc.sbuf_tensor([128, KC, C], dt_rec))
        sb_x = ec(nc.sbuf_tensor([128, NXBUF, IC, CHUNK], dt_gemm))
        sb_xw = ec(nc.sbuf_tensor([128, t_steps, KC, BL], f32))
        sb_h = ec(nc.sbuf_tensor([128, KC, BL], dt_rec))
        sb_h0 = ec(nc.sbuf_tensor([128, KC, BL], dt_rec))
        sb_tmp = ec(nc.sbuf_tensor([128, 2, KC, BL], f32))
        sb_out = ec(nc.sbuf_tensor([C, BL], f32))
        psum_g = [
            ec(nc.psum_tensor(f"pg{i}", [128, CHUNK], f32)) for i in range(NGBANK)
        ]
        # one bank per (step parity, mc-half): the DVE never reads a bank
        # the PE is still writing (P10 hazard)
        psum_r = [
            ec(nc.psum_tensor(f"pr{i}", [128, 2 * BL], f32)) for i in range(4)
        ]
        pfc = ec(nc.psum_tensor("pfc", [C, BL], f32))
        dma_w = ec(nc.semaphore("dma_w"))
        dma_x = [ec(nc.semaphore(f"dma_x{i}")) for i in range(NXBUF)]
        pe_g = ec(nc.semaphore("pe_g"))
        act_g = ec(nc.semaphore("act_g"))
        dve_g = ec(nc.semaphore("dve_g"))
        pe_s = ec(nc.semaphore("pe_s"))
        dve_s = ec(nc.semaphore("dve_s"))
        act_s = ec(nc.semaphore("act_s"))
        fc_s = ec(nc.semaphore("fc_s"))
        block = ec(nc.Block())

        @block.sync
        def _(sync):
            for rep in range(reps):
                if rep > 0:
                    # weight buffers reused: previous rep fully done with them
                    sync.wait_ge(fc_s, rep * TOT_fc_s)
                sync.dma_start(out=sb_WxT[:], in_=WxT[:].rearrange("i p h -> p i h")).then_inc(dma_w, 16)
                sync.dma_start(out=sb_bias[:], in_=bias[:]).then_inc(dma_w, 16)
                sync.dma_start(out=sb_WhT[:], in_=WhT[:].rearrange("k p h -> p k h")).then_inc(dma_w, 16)
                sync.dma_start(out=sb_fcwT[:], in_=fcwT[:].rearrange("k p c -> p k c")).then_inc(dma_w, 16)
                for n in range(nchunks):
                    w = rep * TOT_pe_g + KC * (n - NXBUF + 1)
                    if w > 0:
                        sync.wait_ge(pe_g, w)
                    for ic in range(IC):
                        sync.dma_start(
                            out=sb_x[:, n % NXBUF, ic, :],
                            in_=xT[ic, :, n * CHUNK : (n + 1) * CHUNK],
                        ).then_inc(dma_x[n % NXBUF], 16)
                sync.wait_ge(fc_s, rep * TOT_fc_s + 1)
                sync.dma_start(out=out[:], in_=sb_out[:]).then_inc(dma_w, 16)

        @block.tensor
        def _(tensor):
            for rep in range(reps):
                o_pe = rep * TOT_pe_s
                o_dve = rep * TOT_dve_s
                o_act = rep * TOT_act_s
                # ---- phase 1: input GEMM ----
                tensor.wait_ge(dma_w, rep * TOT_dma_w + 64)
                for n in range(nchunks):
                    tensor.wait_ge(
                        dma_x[n % NXBUF],
                        rep * TOT_dma_x[n % NXBUF] + 32 * (n // NXBUF + 1),
                    )
                    for mc in range(KC):
                        ug = rep * nunits + n * KC + mc
                        if ug >= NGBANK:
                            v = ug - NGBANK
                            gsem = act_g if v % 2 == 0 else dve_g
                            tensor.wait_ge(gsem, v // 2 + 1)
                        pg = psum_g[ug % NGBANK]
                        for ic in range(IC):
                            mm = nc.tensor.matmul(
                                pg[:],
                                sb_WxT[:, ic, mc * 128 : (mc + 1) * 128],
                                sb_x[:, n % NXBUF, ic, :],
                                start=(ic == 0),
                                stop=(ic == IC - 1),
                            )
                        mm.then_inc(pe_g, 1)
                # ---- phase 2: recurrence ----
                tensor.wait_ge(dve_s, o_dve + 1)  # h memset done (per rep)
                for t in range(TS):
                    for q, (mcs, kcs) in enumerate(QUARTERS):
                        if variant == "full":
                            # psum WAR vs DVE(t-2) is transitively implied:
                            # act_s>=2t-1 means ACT(t-1).half0 ran, which
                            # itself waited dve_s>=2t.
                            if q == 0 and t >= 1:
                                tensor.wait_ge(act_s, o_act + 2 * t - 1)
                            elif q == 2 and t >= 1:
                                tensor.wait_ge(act_s, o_act + 2 * t)
                        for mc in mcs:
                            half = mc // 2
                            pr = psum_r[2 * (t % 2) + half][:].rearrange(
                                "p (m b) -> p m b", m=2
                            )
                            rhs_h = sb_h if variant == "full" else sb_h0
                            for kc in kcs:
                                # one accumulation group per bank per step:
                                # start zeroes the whole 2KB zero-region
                                mm = nc.tensor.matmul(
                                    pr[:, mc % 2, :],
                                    sb_WhT[:, kc, mc * 128 : (mc + 1) * 128],
                                    rhs_h[:, kc, :],
                                    start=(kc == 0 and mc in (0, 2) and q in (0, 1)),
                                    stop=(kc == 3 and mc in (1, 3) and q in (2, 3)),
                                    skip_group_check=True,
                                )
                        if q >= 2:
                            # q2: psum half0 ready (implies h0 free);
                            # q3: psum half1 ready + h1 free
                            mm.then_inc(pe_s, 1)
                # ---- phase 3: fc ----
                if rec:
                    tensor.wait_ge(act_s, o_act + 2 * TS)
                else:
                    tensor.wait_ge(act_g, (rep + 1) * TOT_act_g)
                    tensor.wait_ge(dve_g, (rep + 1) * TOT_dve_g)
                    tensor.wait_ge(dve_s, o_dve + 1)  # h memset done
                for jc in range(KC):
                    mm = nc.tensor.matmul(
                        pfc[:],
                        sb_fcwT[:, jc, :],
                        sb_h[:, jc, :],
                        start=(jc == 0),
                        stop=(jc == KC - 1),
                    )
                mm.then_inc(pe_s, 1)

        @block.vector
        def _(vector):
            for rep in range(reps):
                o_pe = rep * TOT_pe_s
                o_dve = rep * TOT_dve_s
                o_act = rep * TOT_act_s
                if rep > 0:
                    vector.wait_ge(pe_s, o_pe)  # prev rep fc done with sb_h
                vector.wait_ge(dma_w, rep * TOT_dma_w + 64)
                for n in range(nchunks):
                    for mc in range(KC):
                        ug = rep * nunits + n * KC + mc
                        if ug % 2 != 1:
                            continue
                        vector.wait_ge(pe_g, ug + 1)
                        pg = psum_g[ug % NGBANK][:].rearrange(
                            "p (t b) -> p t b", b=BL
                        )
                        nc.vector.tensor_scalar_add(
                            sb_xw[:, n * tpc : (n + 1) * tpc, mc, :],
                            pg[:],
                            sb_bias[:, mc : mc + 1],
                        ).then_inc(dve_g, 1)
                nc.vector.memset(sb_h0[:], 0)
                nc.vector.memset(sb_h[:], 0).then_inc(dve_s, 1)
                for t in range(TS):
                    if t % tpc == 0:
                        done = KC * (t // tpc + 1) // 2
                        vector.wait_ge(act_g, rep * TOT_act_g + done)
                        vector.wait_ge(dve_g, rep * TOT_dve_g + done)
                    for half in range(2):
                        pr = psum_r[2 * (t % 2) + half][:].rearrange(
                            "p (m b) -> p m b", m=2
                        )
                        # tmp-WAR vs ACT(t-2) is transitively implied via
                        # pe_s: q2(t) is gated on act_s>=2t, whose ACT ops
                        # waited out DVE(t-1) and hence ACT(t-2).
                        vector.wait_ge(pe_s, o_pe + 2 * t + (1 if half == 0 else 2))
                        ks = slice(2 * half, 2 * half + 2)
                        if variant == "full":
                            nc.vector.tensor_add(
                                sb_tmp[:, t % 2, ks, :],
                                pr[:],
                                sb_xw[:, t, ks, :],
                            ).then_inc(dve_s, 1)
                        else:
                            nc.vector.tensor_copy(
                                sb_tmp[:, t % 2, ks, :],
                                sb_xw[:, t, ks, :],
                            ).then_inc(dve_s, 1)

        @block.scalar
        def _(scalar):
            for rep in range(reps):
                o_pe = rep * TOT_pe_s
                o_dve = rep * TOT_dve_s
                scalar.wait_ge(dma_w, rep * TOT_dma_w + 64)
                for n in range(nchunks):
                    for mc in range(KC):
                        ug = rep * nunits + n * KC + mc
                        if ug % 2 != 0:
                            continue
                        scalar.wait_ge(pe_g, ug + 1)
                        pg = psum_g[ug % NGBANK][:].rearrange(
                            "p (t b) -> p t b", b=BL
                        )
                        nc.scalar.activation(
                            sb_xw[:, n * tpc : (n + 1) * tpc, mc, :],
                            pg[:],
                            mybir.ActivationFunctionType.Identity,
                            bias=sb_bias[:, mc : mc + 1],
                        ).then_inc(act_g, 1)
                for t in range(TS):
                    for half in range(2):
                        # h0-free WAR is implied: dve_s>=2t+2 means
                        # DVE(t).half0 ran, which waited pe_s past q2.
                        scalar.wait_ge(dve_s, o_dve + 2 * t + 2 + half)
                        ks = slice(2 * half, 2 * half + 2)
                        nc.scalar.activation(
                            sb_h[:, ks, :],
                            sb_tmp[:, t % 2, ks, :],
                            mybir.ActivationFunctionType.Tanh,
                        ).then_inc(act_s, 1)
                scalar.wait_ge(pe_s, o_pe + 2 * TS + 1)
                nc.scalar.copy(sb_out[:], pfc[:]).then_inc(fc_s, 1)

    return nc


def _pack_core(x_bt, Wx_w, Wx_b, Wh_w, Wh_b, fcw_slice, np_gemm, np_rec,
               t_steps=T):
    """Host-side layout prep for one core. x_bt: [BL, T, I] (already
    time-reversed for backward cores)."""
    xT = np.ascontiguousarray(x_bt.transpose(2, 1, 0)).reshape(IC, 128, t_steps * BL)
    WxT = np.ascontiguousarray(Wx_w.T).reshape(IC, 128, H)
    WhT = np.ascontiguousarray(Wh_w.T).reshape(KC, 128, H)
    b = np.ascontiguousarray((Wx_b + Wh_b).astype(np.float32).reshape(KC, 128).T)
    fcwT = np.ascontiguousarray(fcw_slice.T).reshape(KC, 128, C)
    return {
        "xT": xT.astype(np_gemm),
        "WxT": WxT.astype(np_gemm),
        "WhT": WhT.astype(np_rec),
        "bias": b,
        "fcwT": fcwT.astype(np_rec),
    }


_NC_CACHE = {}


def make_in_maps(x, Wxf_w, Wxf_b, Whf_w, Whf_b, Wxb_w, Wxb_b, Whb_w, Whb_b,
                 fc_w, np_gemm=np.float32, np_rec=np.float32, t_steps=T):
    in_maps = []
    for core in range(NCORES):
        fwd = core < 4
        g = core % 4
        bs = slice(g * BL, (g + 1) * BL)
        if fwd:
            xs = x[bs, :t_steps]
            m = _pack_core(xs, Wxf_w, Wxf_b, Whf_w, Whf_b, fc_w[:, :H],
                           np_gemm, np_rec, t_steps)
        else:
            xs = x[bs, :t_steps][:, ::-1]
            m = _pack_core(xs, Wxb_w, Wxb_b, Whb_w, Whb_b, fc_w[:, H:],
                           np_gemm, np_rec, t_steps)
        in_maps.append(m)
    return in_maps


import ml_dtypes

DT_GEMM = mybir.dt.bfloat16
DT_REC = mybir.dt.bfloat16
NP_GEMM = ml_dtypes.bfloat16
NP_REC = ml_dtypes.bfloat16


def _run(x, Wxf_w, Wxf_b, Whf_w, Whf_b, Wxb_w, Wxb_b, Whb_w, Whb_b, fc_w, fc_b,
         trace=False, **trace_kwargs):
    key = ("nc", T, DT_GEMM, DT_REC)
    if key not in _NC_CACHE:
        _NC_CACHE[key] = build_nc(T, DT_GEMM, DT_REC)
    nc = _NC_CACHE[key]
    in_maps = make_in_maps(x, Wxf_w, Wxf_b, Whf_w, Whf_b, Wxb_w, Wxb_b,
                           Whb_w, Whb_b, fc_w, NP_GEMM, NP_REC)
    res = run_bass_kernel_spmd(nc, in_maps, list(range(NCORES)), trace=trace,
                               **trace_kwargs)
    out = np.zeros((B, C), np.float32)
    for g in range(4):
        out[g * BL : (g + 1) * BL] = (
            res.results[g]["out"].T + res.results[4 + g]["out"].T
        )
    out += fc_b[None, :]
    return out, res


def kernel(x, Wxf_w, Wxf_b, Whf_w, Whf_b, Wxb_w, Wxb_b, Whb_w, Whb_b, fc_w, fc_b):
    out, _ = _run(x, Wxf_w, Wxf_b, Whf_w, Whf_b, Wxb_w, Wxb_b, Whb_w, Whb_b,
                  fc_w, fc_b)
    return out


def bench_in_maps(inputs):
    a = {k: v for k, v in inputs.items() if k != "fc_b"}
    return make_in_maps(**a, np_gemm=NP_GEMM, np_rec=NP_REC)

